# revision 12
# baseline (speedup 1.0000x reference)
"""CharRNN (2-layer LSTM + vocab projection) on 8 trn2 NeuronCores.

Strategy:
  - LSTM gate-sharded 8-way: core j owns hidden slice [128j:128j+128] of both
    layers (all 4 gates for that slice). Weights stay SBUF-resident in bf16.
  - Layer 2 runs one timestep behind layer 1; each produced h-slice is
    AllGathered (bf16, [128,32] per rank) so every core has the full h.T for
    the next step's matmuls. Everything is gate-major ([hidden, batch]) so no
    transposes are ever needed and H2.T accumulates directly as the stationary
    operand for the projection.
  - Projection vocab-sharded: core j computes logits[:, 4000j:4000j+4000].
"""

import sys

sys.path.insert(0, "/opt/trn_rl_repo")

import ml_dtypes
import numpy as np

import concourse.bass as bass
import concourse.tile as tile
from concourse import bacc, mybir
from concourse.bass_utils import run_bass_kernel_spmd

V, B, E, H = 32000, 32, 256, 1024
NCORES = 8
VSH = V // NCORES  # 4000
GSH = 4 * (H // NCORES)  # 512 gate cols per core (4 gates x 128)
F32 = mybir.dt.float32
BF16 = mybir.dt.bfloat16
SIG = mybir.ActivationFunctionType.Sigmoid
TANH = mybir.ActivationFunctionType.Tanh

_CACHE = {}


def _build(T, debug=False):
    BT = B * T
    KX = E // 128  # 2
    KH = H // 128  # 8
    K1 = KX + KH  # 10
    K2 = 2 * KH  # 16
    MCH = BT // 128
    NVC = 500  # projection free-dim chunk
    NV = VSH // NVC  # 8

    nc = bacc.Bacc("TRN2", target_bir_lowering=False, debug=False, num_devices=NCORES)

    xT_d = nc.dram_tensor("xT", [E, BT], BF16, kind="ExternalInput")
    w1_d = nc.dram_tensor("w1s", [E + H, GSH], BF16, kind="ExternalInput")
    w2_d = nc.dram_tensor("w2s", [2 * H, GSH], BF16, kind="ExternalInput")
    b1_d = nc.dram_tensor("b1s", [GSH], F32, kind="ExternalInput")
    b2_d = nc.dram_tensor("b2s", [GSH], F32, kind="ExternalInput")
    swT_d = nc.dram_tensor("swTs", [H, VSH], BF16, kind="ExternalInput")
    out_d = nc.dram_tensor("out", [BT, VSH], F32, kind="ExternalOutput")
    if debug:
        dz1_d = nc.dram_tensor("dz1", [128, 4, 32], F32, kind="ExternalOutput")
        dh1b_d = nc.dram_tensor("dh1b", [128, 32], F32, kind="ExternalOutput")
        dh1T_d = nc.dram_tensor("dh1T", [128, KH, 32], F32, kind="ExternalOutput")
        dh2T_d = nc.dram_tensor("dh2T", [128, KH, BT], F32, kind="ExternalOutput")

    rg = [list(range(NCORES))]

    with tile.TileContext(nc) as tc:
        with (
            tc.tile_pool(name="const", bufs=1) as cpool,
            tc.tile_pool(name="state", bufs=1) as spool,
            tc.tile_pool(name="gates", bufs=3) as gpool,
            tc.tile_pool(name="z1p", bufs=2, space="PSUM") as z1pool,
            tc.tile_pool(name="z2p", bufs=2, space="PSUM") as z2pool,
            tc.tile_pool(name="pp", bufs=1, space="PSUM") as ppool,
            tc.tile_pool(name="dram", bufs=3, space="DRAM") as dpool,
            tc.tile_pool(name="outb", bufs=4) as opool,
        ):
            # ---- resident tensors ----
            w1 = cpool.tile([128, K1, GSH], BF16)
            nc.sync.dma_start(w1[:], w1_d.ap().rearrange("(k p) m -> p k m", p=128))
            w2 = cpool.tile([128, K2, GSH], BF16)
            nc.sync.dma_start(w2[:], w2_d.ap().rearrange("(k p) m -> p k m", p=128))
            xt = cpool.tile([128, KX, BT], BF16)
            nc.sync.dma_start(xt[:], xT_d.ap().rearrange("(k p) n -> p k n", p=128))
            swt = cpool.tile([128, KH, VSH], BF16)
            nc.sync.dma_start(swt[:], swT_d.ap().rearrange("(k p) n -> p k n", p=128))
            b1s = cpool.tile([128, 4], F32)
            nc.sync.dma_start(b1s[:], b1_d.ap().rearrange("(c p) -> p c", p=128))
            b2s = cpool.tile([128, 4], F32)
            nc.sync.dma_start(b2s[:], b2_d.ap().rearrange("(c p) -> p c", p=128))

            H2T = cpool.tile([128, KH, BT], BF16)

            h1T = spool.tile([128, KH, 32], BF16)
            h2T = spool.tile([128, KH, 32], BF16)
            c1 = spool.tile([128, 32], F32)
            c2 = spool.tile([128, 32], F32)
            nc.gpsimd.memset(h1T[:], 0.0)
            nc.gpsimd.memset(h2T[:], 0.0)
            nc.gpsimd.memset(c1[:], 0.0)
            nc.gpsimd.memset(c2[:], 0.0)

            def gates(zp, bsb, c_st, tag):
                """LSTM cell update from gate pre-activations zp [128,4,32].

                m-chunks: 0=i 1=j 2=f 3=o. Updates c_st in place, returns
                bf16 h tile."""
                ig = gpool.tile([128, 32], F32, tag=f"ig{tag}")
                jg = gpool.tile([128, 32], F32, tag=f"jg{tag}")
                fg = gpool.tile([128, 32], F32, tag=f"fg{tag}")
                og = gpool.tile([128, 32], F32, tag=f"og{tag}")
                nc.scalar.activation(ig[:], zp[:, 0, :], SIG, bias=bsb[:, 0:1])
                nc.scalar.activation(jg[:], zp[:, 1, :], TANH, bias=bsb[:, 1:2])
                nc.scalar.activation(fg[:], zp[:, 2, :], SIG, bias=bsb[:, 2:3])
                nc.scalar.activation(og[:], zp[:, 3, :], SIG, bias=bsb[:, 3:4])
                nc.vector.tensor_mul(c_st[:], c_st[:], fg[:])
                tmp = gpool.tile([128, 32], F32, tag=f"tmp{tag}")
                nc.vector.tensor_mul(tmp[:], ig[:], jg[:])
                nc.vector.tensor_add(c_st[:], c_st[:], tmp[:])
                th = gpool.tile([128, 32], F32, tag=f"th{tag}")
                nc.scalar.activation(th[:], c_st[:], TANH)
                hb = gpool.tile([128, 32], BF16, tag=f"hb{tag}")
                nc.vector.tensor_mul(hb[:], th[:], og[:])
                return hb

            # ---- projection pass machinery (interleaved into the loop so
            # its matmuls fill the AllGather-wait gaps) ----
            out_v = out_d.ap().rearrange("(b t) v -> t b v", t=T)
            proj_passes = [
                (m, nh, k)
                for m in range(MCH)
                for nh in range(NV // 4)
                for k in range(KH)
            ]
            proj_state = {}  # (m, nh) -> psum tiles
            proj_emitted = 0

            def emit_proj_pass(m, nh, k):
                key = (m, nh)
                if key not in proj_state:
                    proj_state[key] = [
                        ppool.tile([128, NVC], F32, tag=f"pp{i}", name=f"pp{i}")
                        for i in range(4)
                    ]
                pps = proj_state[key]
                for ni in range(4):
                    n = nh * 4 + ni
                    nc.tensor.matmul(
                        pps[ni][:],
                        H2T[:, k, bass.ts(m, 128)],
                        swt[:, k, bass.ds(n * NVC, NVC)],
                        start=(k == 0),
                        stop=(k == KH - 1),
                    )
                if k == KH - 1:
                    for ni in range(4):
                        n = nh * 4 + ni
                        ob = opool.tile([128, NVC], F32, tag="ob", name="ob")
                        nc.vector.tensor_copy(ob[:], pps[ni][:])
                        t0 = m * 4
                        for tt in range(4):
                            nc.sync.dma_start(
                                out_v[t0 + tt, :, n * NVC : (n + 1) * NVC],
                                ob[32 * tt : 32 * tt + 32, :],
                            )
                    del proj_state[key]

            # ---- recurrence, layer 2 skewed one step behind layer 1 ----
            for s in range(T + 1):
                t1 = s
                t2 = s - 1
                ag1o = ag2o = None
                if t1 < T:
                    z1 = z1pool.tile([128, 4, 32], F32)
                    for c in range(4):
                        for k in range(K1):
                            rhs = (
                                xt[:, k, bass.ts(t1, 32)]
                                if k < KX
                                else h1T[:, k - KX, :]
                            )
                            nc.tensor.matmul(
                                z1[:, c, :],
                                w1[:, k, bass.ts(c, 128)],
                                rhs,
                                start=(k == 0),
                                stop=(k == K1 - 1),
                            )
                    h1b = gates(z1, b1s, c1, "a")
                    if debug and s == 0:
                        zb = opool.tile([128, 4, 32], F32, tag="dbg1")
                        nc.vector.tensor_copy(zb[:], z1[:])
                        nc.sync.dma_start(dz1_d.ap(), zb[:])
                        hb32 = opool.tile([128, 32], F32, tag="dbg2")
                        nc.vector.tensor_copy(hb32[:], h1b[:])
                        nc.sync.dma_start(dh1b_d.ap(), hb32[:])
                    ag1i = dpool.tile([128, 32], BF16, tag="ag1i")
                    nc.sync.dma_start(ag1i[:], h1b[:])
                    ag1o = dpool.tile([NCORES * 128, 32], BF16, tag="ag1o")
                    nc.gpsimd.collective_compute(
                        "AllGather",
                        mybir.AluOpType.bypass,
                        replica_groups=rg,
                        ins=[ag1i.opt()],
                        outs=[ag1o.opt()],
                    )
                if t2 >= 0:
                    z2 = z2pool.tile([128, 4, 32], F32)
                    for c in range(4):
                        for k in range(K2):
                            rhs = h1T[:, k, :] if k < KH else h2T[:, k - KH, :]
                            nc.tensor.matmul(
                                z2[:, c, :],
                                w2[:, k, bass.ts(c, 128)],
                                rhs,
                                start=(k == 0),
                                stop=(k == K2 - 1),
                            )
                    h2b = gates(z2, b2s, c2, "b")
                    ag2i = dpool.tile([128, 32], BF16, tag="ag2i")
                    nc.sync.dma_start(ag2i[:], h2b[:])
                    ag2o = dpool.tile([NCORES * 128, 32], BF16, tag="ag2o")
                    nc.gpsimd.collective_compute(
                        "AllGather",
                        mybir.AluOpType.bypass,
                        replica_groups=rg,
                        ins=[ag2i.opt()],
                        outs=[ag2o.opt()],
                    )
                # write gathered state back (Tile serializes after the reads above)
                if ag1o is not None:
                    nc.sync.dma_start(
                        h1T[:], ag1o.opt().rearrange("(k p) n -> p k n", p=128)
                    )
                    if debug and s == 0:
                        ht32 = opool.tile([128, KH, 32], F32, tag="dbg3")
                        nc.vector.tensor_copy(ht32[:], h1T[:])
                        nc.sync.dma_start(dh1T_d.ap(), ht32[:])
                if ag2o is not None:
                    gathered = ag2o.opt().rearrange("(k p) n -> p k n", p=128)
                    nc.sync.dma_start(h2T[:], gathered)
                    nc.sync.dma_start(H2T[:, :, bass.ts(t2, 32)], gathered)

                # interleave projection passes whose H2T columns are ready
                # (m-chunk m covers t2 in [4m, 4m+4), gathered by step 4m+4)
                if s >= 4:
                    budget = proj_emitted + 5
                    while (
                        proj_emitted < len(proj_passes)
                        and proj_emitted < budget
                        and proj_passes[proj_emitted][0] <= s // 4 - 1
                    ):
                        emit_proj_pass(*proj_passes[proj_emitted])
                        proj_emitted += 1

            if debug:
                h2all = opool.tile([128, KH, BT], F32, tag="dbg4")
                nc.vector.tensor_copy(h2all[:], H2T[:])
                nc.sync.dma_start(dh2T_d.ap(), h2all[:])

            # ---- drain any remaining projection passes ----
            while proj_emitted < len(proj_passes):
                emit_proj_pass(*proj_passes[proj_emitted])
                proj_emitted += 1

    nc.compile()
    return nc


def _prep_inputs(input_data, embedding, W1, b1, W2, b2, softmax_w, softmax_b):
    T = input_data.shape[1]
    ids = np.asarray(input_data).astype(np.int64)
    x = np.asarray(embedding, np.float32)[ids]  # [B, T, E]
    xT = np.ascontiguousarray(x.transpose(2, 1, 0).reshape(E, T * B))
    xT_bf = xT.astype(ml_dtypes.bfloat16)

    in_maps = []
    for j in range(NCORES):
        sl = [slice(g * H + 128 * j, g * H + 128 * j + 128) for g in range(4)]
        w1s = np.concatenate([W1[:, s] for s in sl], axis=1)
        w2s = np.concatenate([W2[:, s] for s in sl], axis=1)
        b1s = np.concatenate([b1[s] for s in sl])
        b2s = np.concatenate([b2[s] for s in sl])
        swTs = np.ascontiguousarray(softmax_w[j * VSH : (j + 1) * VSH, :].T)
        in_maps.append(
            {
                "xT": xT_bf,
                "w1s": np.ascontiguousarray(w1s).astype(ml_dtypes.bfloat16),
                "w2s": np.ascontiguousarray(w2s).astype(ml_dtypes.bfloat16),
                "b1s": np.ascontiguousarray(b1s).astype(np.float32),
                "b2s": np.ascontiguousarray(b2s).astype(np.float32),
                "swTs": swTs.astype(ml_dtypes.bfloat16),
            }
        )
    return in_maps


def kernel(input_data, embedding, W1, b1, W2, b2, softmax_w, softmax_b, **kw):
    T = input_data.shape[1]
    if T not in _CACHE:
        _CACHE[T] = _build(T)
    nc = _CACHE[T]
    in_maps = _prep_inputs(
        input_data, embedding, W1, b1, W2, b2, softmax_w, softmax_b
    )
    res = run_bass_kernel_spmd(nc, in_maps, core_ids=list(range(NCORES)), **kw)
    logits = np.concatenate([r["out"] for r in res.results], axis=1)
    sb = np.asarray(softmax_b, np.float32)
    if np.any(sb):
        logits = logits + sb[None, :]
    return logits.astype(np.float32)


if __name__ == "__main__":
    rng = np.random.default_rng(0)
    T = 8
    inputs = {
        "input_data": rng.integers(0, V, (B, T)).astype(np.int64),
        "embedding": rng.standard_normal((V, E), np.float32) * 0.01,
        "W1": rng.standard_normal((E + H, 4 * H), np.float32) * 0.05,
        "b1": np.zeros(4 * H, np.float32),
        "W2": rng.standard_normal((2 * H, 4 * H), np.float32) * 0.05,
        "b2": np.zeros(4 * H, np.float32),
        "softmax_w": (rng.standard_normal((V, H), np.float32) / 32.0),
        "softmax_b": np.zeros(V, np.float32),
    }
    out = kernel(**inputs)
    print("kernel out", out.shape, out.dtype, float(np.abs(out).max()))
